# revision 2
# baseline (speedup 1.0000x reference)
"""Batched indirect-DMA embedding kernel.

Layout: each core handles nshard=131072 tokens, split into nblk blocks of
P*T tokens (T tokens per partition, partition-major order). Per block:

  h(b):    ONE indirect gather of H[x] rows: offsets x_sb[:, bT:(b+1)T]
           ([P, T] indices), each index pulls a 16-int32 row -> h_sb slot.
  t0(b):   ONE indirect slice gather from table0: offsets h_sb[.., 0:8]
           ([P, T, 8] indices), each pulls 8 consecutive f32 -> o_sb slot.
  t1(b):   same from table1 with CCE-add accumulate into o_sb slot.
  store(b): ONE contiguous store of [P, T*64] f32 (T*256B per partition).

vs. the per-128-token-block version (17 indirect DMAs/block), this issues
3 Pool DMAs per P*T tokens, amortizing the ~1us SWDGE per-instruction
overhead and using large per-partition descriptors.
"""

import numpy as np

VOCAB = 1_000_000
SIZE = 262_144
CHUNK = 8
NCHUNKS = 8
N = 1_048_576
DIM = CHUNK * NCHUNKS

NCORES = 8
NSHARD = N // NCORES  # 131072
P = 128
T = 64  # tokens per partition per block
HBUF = 4
OBUF = 4


def build_kernel(nshard=NSHARD, t=T):
    import concourse.bass as bass
    import concourse.mybir as mybir
    from concourse.bass import IndirectOffsetOnAxis
    import contextlib

    nblk = nshard // (P * t)
    assert nblk * P * t == nshard
    nc = bass.Bass(trn_type="TRN2")
    # host passes x partition-major: x_w[p, b*t + j] = x[b*P*t + p*t + j]
    x_t = nc.dram_tensor("x", [P, nblk * t], mybir.dt.int32, kind="ExternalInput")
    h_t = nc.dram_tensor(
        "h", [VOCAB, 2 * NCHUNKS], mybir.dt.int32, kind="ExternalInput"
    )
    t0_t = nc.dram_tensor(
        "t0", [SIZE + CHUNK, 1], mybir.dt.float32, kind="ExternalInput"
    )
    t1_t = nc.dram_tensor(
        "t1", [SIZE + CHUNK, 1], mybir.dt.float32, kind="ExternalInput"
    )
    out_t = nc.dram_tensor(
        "out", [nshard, DIM], mybir.dt.float32, kind="ExternalOutput"
    )

    # [nblk, P, t*64]; row b*P*t + p*t + j <- o_sb[p, j*64:(j+1)*64]
    out_v = out_t[:].rearrange("(b p t) d -> b p (t d)", p=P, t=t)

    with contextlib.ExitStack() as ctx:
        x_sb = ctx.enter_context(
            nc.sbuf_tensor("x_sb", [P, nblk * t], mybir.dt.int32)
        )
        h_sb = ctx.enter_context(
            nc.sbuf_tensor("h_sb", [P, HBUF * t, 16], mybir.dt.int32)
        )
        o_sb = ctx.enter_context(
            nc.sbuf_tensor("o_sb", [P, OBUF, t * DIM], mybir.dt.float32)
        )
        sem_x = ctx.enter_context(nc.semaphore("sem_x"))
        sem_h = [ctx.enter_context(nc.semaphore(f"sem_h{s}")) for s in range(HBUF)]
        sem_t0 = [ctx.enter_context(nc.semaphore(f"sem_t0{s}")) for s in range(OBUF)]
        sem_t1 = [ctx.enter_context(nc.semaphore(f"sem_t1{s}")) for s in range(OBUF)]
        sem_st = [ctx.enter_context(nc.semaphore(f"sem_st{s}")) for s in range(OBUF)]

        nc.sync.dma_start(x_sb[:], x_t[:]).then_inc(sem_x, 16)

        for L in range(nblk + 3):
            # ---- Pool: h-row gather for block L ----
            if L < nblk:
                if L == 0:
                    nc.gpsimd.wait_ge(sem_x, 16)
                if L >= HBUF:
                    # h slot reuse: t1 of block L-HBUF was the last reader
                    k = L - HBUF
                    nc.gpsimd.wait_ge(sem_t1[k % OBUF], 16 * (k // OBUF + 1))
                s = L % HBUF
                nc.gpsimd.indirect_dma_start(
                    out=h_sb[:, s * t : (s + 1) * t, :],
                    out_offset=None,
                    in_=h_t[:],
                    in_offset=IndirectOffsetOnAxis(
                        ap=x_sb[:, L * t : (L + 1) * t], axis=0
                    ),
                ).then_inc(sem_h[s], 16)

            # ---- Pool: table0 slice gather for block b0 = L-1 ----
            b0 = L - 1
            if 0 <= b0 < nblk:
                nc.gpsimd.wait_ge(sem_h[b0 % HBUF], 16 * (b0 // HBUF + 1))
                if b0 >= OBUF:
                    # o slot reuse: store of block b0-OBUF must be done
                    k = b0 - OBUF
                    nc.gpsimd.wait_ge(sem_st[k % OBUF], 16 * (k // OBUF + 1))
                s = b0 % HBUF
                nc.gpsimd.indirect_dma_start(
                    out=o_sb[:, b0 % OBUF, :],
                    out_offset=None,
                    in_=t0_t[:],
                    in_offset=IndirectOffsetOnAxis(
                        ap=h_sb[:, s * t : (s + 1) * t, 0:NCHUNKS], axis=0
                    ),
                ).then_inc(sem_t0[b0 % OBUF], 16)

            # ---- Pool: table1 slice gather + CCE add for block b1 = L-2 ----
            b1 = L - 2
            if 0 <= b1 < nblk:
                nc.gpsimd.wait_ge(sem_t0[b1 % OBUF], 16 * (b1 // OBUF + 1))
                s = b1 % HBUF
                nc.gpsimd.indirect_dma_start(
                    out=o_sb[:, b1 % OBUF, :],
                    out_offset=None,
                    in_=t1_t[:],
                    in_offset=IndirectOffsetOnAxis(
                        ap=h_sb[:, s * t : (s + 1) * t, NCHUNKS : 2 * NCHUNKS],
                        axis=0,
                    ),
                    compute_op=mybir.AluOpType.add,
                ).then_inc(sem_t1[b1 % OBUF], 16)

            # ---- SP: store block L-3 ----
            sb = L - 3
            if 0 <= sb < nblk:
                nc.sync.wait_ge(sem_t1[sb % OBUF], 16 * (sb // OBUF + 1))
                nc.sync.dma_start(out_v[sb], o_sb[:, sb % OBUF, :]).then_inc(
                    sem_st[sb % OBUF], 16
                )

        for s in range(OBUF):
            ns = len([k for k in range(nblk) if k % OBUF == s])
            if ns:
                nc.sync.wait_ge(sem_st[s], ns * 16)
    return nc


def prep_inputs(table0, table1, h0, h1, x):
    x = np.ascontiguousarray(x.astype(np.int32))
    nblk = NSHARD // (P * T)
    # [N] -> per-core partition-major layouts [NCORES, P, nblk*T]
    xs = x.reshape(NCORES, nblk, P, T)
    xw = np.ascontiguousarray(np.transpose(xs, (0, 2, 1, 3))).reshape(
        NCORES, P, nblk * T
    )
    H = np.ascontiguousarray(np.concatenate([h0, h1], axis=1).astype(np.int32))
    t0 = np.ascontiguousarray(
        np.concatenate([table0, table0[:CHUNK]]).astype(np.float32)
    ).reshape(SIZE + CHUNK, 1)
    t1 = np.ascontiguousarray(
        np.concatenate([table1, table1[:CHUNK]]).astype(np.float32)
    ).reshape(SIZE + CHUNK, 1)
    return xw, H, t0, t1


def kernel(table0, table1, h0, h1, x):
    from concourse.bass_utils import run_bass_kernel_spmd

    xw, H, t0, t1 = prep_inputs(table0, table1, h0, h1, x)
    nc = build_kernel()
    in_maps = [
        {"x": xw[k], "h": H, "t0": t0, "t1": t1} for k in range(NCORES)
    ]
    res = run_bass_kernel_spmd(nc, in_maps, core_ids=list(range(NCORES)))
    return np.concatenate([r["out"] for r in res.results], axis=0)


# revision 3
# speedup vs baseline: 80.8946x; 80.8946x over previous
"""Batched indirect-DMA embedding kernel.

Layout: each core handles nshard=131072 tokens, split into nblk blocks of
P*T tokens (T tokens per partition, partition-major order). Per block:

  h(b):    ONE indirect gather of H[x] rows: offsets x_sb[:, bT:(b+1)T]
           ([P, T] indices, pre-scaled by 16 on host), each index pulls
           16 consecutive int32 (one packed h0|h1 row) -> h_sb slot.
  t0(b):   ONE indirect slice gather from table0: offsets h_sb[.., 0:8]
           ([P, T, 8] indices), each pulls 8 consecutive f32 -> o_sb slot.
  t1(b):   same from table1 with CCE-add accumulate into o_sb slot.
  store(b): ONE contiguous store of [P, T*64] f32 (T*256B per partition).

DRAM tensors are declared with flat [1, n] shapes and gathers use axis=1
offsets so the cost/descriptor math sees large contiguous last dims; SBUF
gather destinations are 2-D slices (offset views are strided rearranges
of the same buffer).
"""

import numpy as np

VOCAB = 1_000_000
SIZE = 262_144
CHUNK = 8
NCHUNKS = 8
N = 1_048_576
DIM = CHUNK * NCHUNKS

NCORES = 8
NSHARD = N // NCORES  # 131072
P = 128
T = 64  # tokens per partition per block
HBUF = 4
OBUF = 4


def build_kernel(nshard=NSHARD, t=T):
    import concourse.bass as bass
    import concourse.mybir as mybir
    from concourse.bass import IndirectOffsetOnAxis
    import contextlib

    nblk = nshard // (P * t)
    assert nblk * P * t == nshard
    hrow = 2 * NCHUNKS  # 16 ints per packed h row
    nc = bass.Bass(trn_type="TRN2")
    # host passes x partition-major and pre-scaled by 16:
    # x_w[p, b*t + j] = 16 * x[b*P*t + p*t + j]
    x_t = nc.dram_tensor("x", [P, nblk * t], mybir.dt.int32, kind="ExternalInput")
    h_t = nc.dram_tensor("h", [1, VOCAB * hrow], mybir.dt.int32, kind="ExternalInput")
    t0_t = nc.dram_tensor("t0", [1, SIZE + CHUNK], mybir.dt.float32, kind="ExternalInput")
    t1_t = nc.dram_tensor("t1", [1, SIZE + CHUNK], mybir.dt.float32, kind="ExternalInput")
    out_t = nc.dram_tensor("out", [nshard, DIM], mybir.dt.float32, kind="ExternalOutput")

    # [nblk, P, t*64]; row b*P*t + p*t + j <- o_sb[p, j*64:(j+1)*64]
    out_v = out_t[:].rearrange("(b p t) d -> b p (t d)", p=P, t=t)

    with contextlib.ExitStack() as ctx:
        x_sb = ctx.enter_context(
            nc.sbuf_tensor("x_sb", [P, nblk * t], mybir.dt.int32)
        )
        h_sb = ctx.enter_context(
            nc.sbuf_tensor("h_sb", [P, HBUF * t * hrow], mybir.dt.int32)
        )
        o_sb = ctx.enter_context(
            nc.sbuf_tensor("o_sb", [P, OBUF, t * DIM], mybir.dt.float32)
        )
        sem_x = ctx.enter_context(nc.semaphore("sem_x"))
        sem_h = [ctx.enter_context(nc.semaphore(f"sem_h{s}")) for s in range(HBUF)]
        sem_t0 = [ctx.enter_context(nc.semaphore(f"sem_t0{s}")) for s in range(OBUF)]
        sem_t1 = [ctx.enter_context(nc.semaphore(f"sem_t1{s}")) for s in range(OBUF)]
        sem_st = [ctx.enter_context(nc.semaphore(f"sem_st{s}")) for s in range(OBUF)]

        def h_flat(s):
            # gather-out view: [P, t*16] contiguous slot s
            return h_sb[:, s * t * hrow : (s + 1) * t * hrow]

        def h_idx(s, half):
            # offset view: [P, t, 8] indices for table{half}
            return h_flat(s).rearrange("p (t c) -> p t c", c=hrow)[
                :, :, half * NCHUNKS : (half + 1) * NCHUNKS
            ]

        nc.sync.dma_start(x_sb[:], x_t[:]).then_inc(sem_x, 16)

        for L in range(nblk + 3):
            # ---- Pool: h-row gather for block L ----
            if L < nblk:
                if L == 0:
                    nc.gpsimd.wait_ge(sem_x, 16)
                if L >= HBUF:
                    # h slot reuse: t1 of block L-HBUF was the last reader
                    k = L - HBUF
                    nc.gpsimd.wait_ge(sem_t1[k % OBUF], 16 * (k // OBUF + 1))
                nc.gpsimd.indirect_dma_start(
                    out=h_flat(L % HBUF),
                    out_offset=None,
                    in_=h_t[:],
                    in_offset=IndirectOffsetOnAxis(
                        ap=x_sb[:, L * t : (L + 1) * t], axis=1
                    ),
                ).then_inc(sem_h[L % HBUF], 16)

            # ---- Pool: table0 slice gather for block b0 = L-1 ----
            b0 = L - 1
            if 0 <= b0 < nblk:
                nc.gpsimd.wait_ge(sem_h[b0 % HBUF], 16 * (b0 // HBUF + 1))
                if b0 >= OBUF:
                    # o slot reuse: store of block b0-OBUF must be done
                    k = b0 - OBUF
                    nc.gpsimd.wait_ge(sem_st[k % OBUF], 16 * (k // OBUF + 1))
                nc.gpsimd.indirect_dma_start(
                    out=o_sb[:, b0 % OBUF, :],
                    out_offset=None,
                    in_=t0_t[:],
                    in_offset=IndirectOffsetOnAxis(ap=h_idx(b0 % HBUF, 0), axis=1),
                ).then_inc(sem_t0[b0 % OBUF], 16)

            # ---- Pool: table1 slice gather + CCE add for block b1 = L-2 ----
            b1 = L - 2
            if 0 <= b1 < nblk:
                nc.gpsimd.wait_ge(sem_t0[b1 % OBUF], 16 * (b1 // OBUF + 1))
                nc.gpsimd.indirect_dma_start(
                    out=o_sb[:, b1 % OBUF, :],
                    out_offset=None,
                    in_=t1_t[:],
                    in_offset=IndirectOffsetOnAxis(ap=h_idx(b1 % HBUF, 1), axis=1),
                    compute_op=mybir.AluOpType.add,
                ).then_inc(sem_t1[b1 % OBUF], 16)

            # ---- SP: store block L-3 ----
            sb = L - 3
            if 0 <= sb < nblk:
                nc.sync.wait_ge(sem_t1[sb % OBUF], 16 * (sb // OBUF + 1))
                nc.sync.dma_start(out_v[sb], o_sb[:, sb % OBUF, :]).then_inc(
                    sem_st[sb % OBUF], 16
                )

        for s in range(OBUF):
            ns = len([k for k in range(nblk) if k % OBUF == s])
            if ns:
                nc.sync.wait_ge(sem_st[s], ns * 16)
    return nc


def prep_inputs(table0, table1, h0, h1, x):
    x = np.ascontiguousarray(x.astype(np.int32))
    nblk = NSHARD // (P * T)
    # [N] -> per-core partition-major layouts [NCORES, P, nblk*T], x16 so the
    # flat h gather (coef=1) lands on 16-int32 row starts
    xs = x.reshape(NCORES, nblk, P, T)
    xw = np.ascontiguousarray(np.transpose(xs, (0, 2, 1, 3))).reshape(
        NCORES, P, nblk * T
    ) * np.int32(2 * NCHUNKS)
    H = np.ascontiguousarray(
        np.concatenate([h0, h1], axis=1).astype(np.int32)
    ).reshape(1, VOCAB * 2 * NCHUNKS)
    t0 = np.ascontiguousarray(
        np.concatenate([table0, table0[:CHUNK]]).astype(np.float32)
    ).reshape(1, SIZE + CHUNK)
    t1 = np.ascontiguousarray(
        np.concatenate([table1, table1[:CHUNK]]).astype(np.float32)
    ).reshape(1, SIZE + CHUNK)
    return xw, H, t0, t1


def kernel(table0, table1, h0, h1, x):
    from concourse.bass_utils import run_bass_kernel_spmd

    xw, H, t0, t1 = prep_inputs(table0, table1, h0, h1, x)
    nc = build_kernel()
    in_maps = [
        {"x": xw[k], "h": H, "t0": t0, "t1": t1} for k in range(NCORES)
    ]
    res = run_bass_kernel_spmd(nc, in_maps, core_ids=list(range(NCORES)))
    return np.concatenate([r["out"] for r in res.results], axis=0)
